# revision 13
# baseline (speedup 1.0000x reference)
"""Trainium2 Bass kernel for nn_ContrastiveLoss (B=2048, D=1024, 8 cores), v2.

Math: with G the [3B,3B] cosine-sim Gram of the normalized embeddings,
per ordered pair ell = y*(1-g)^2 + (1-y)*relu(g-1/2)^2 = y*M + A - M
where A = (1-g)^2, R2 = relu(g-1/2)^2, M = A - R2, y = (L_r == L_s).
loss = sum_upper ell' / P with ell' = ell/2 -> loss = S_ordered/(4P).

Device strategy (SPMD, rotation-symmetric triangle, 8 cores):
  - host ships each core its X^T slab [D, 768] bf16 + one-hot labels.
  - each core normalizes its slab, casts to fp8e4, and remote-DMA's it
    (SBUF->SBUF) to cores j+1..j+4 (mod 8); slabs land in slot s =
    sender distance.  Every unordered slab pair {a,b} is covered once:
    distance d=1..3 blocks are computed in full at weight 2 by the
    lower core, d=4 and own-diagonal blocks as weighted triangles
    (mask W = 2 above / 1 on / 0 below the offset diagonal).
  - gram tiles [128,384] via fp8 DoubleRow matmuls (2 k-tiles/inst),
    elementwise post-processing split across Scalar/Vector/GpSimd:
      A  = Square(-g+1)  (+row-sum accum)   [Scalar]
      r  = (g max .5) - .5                  [Vector]
      R2 = r*r                              [Vector]
      M  = A - R2 (bf16)                    [GpSimd]
    per-class column sums of M via u^T @ M matmuls (u = one-hot*w).
  - host: S = sum_cls/col masked accM + sum w*Asums - sum accM, /(4P).
"""

import sys
import numpy as np

for _p in ("/opt/trn_rl_repo",):
    if _p not in sys.path:
        sys.path.insert(0, _p)

import ml_dtypes  # noqa: E402

import concourse.bass as bass  # noqa: E402
import concourse.bacc as bacc  # noqa: E402
import concourse.tile as tile  # noqa: E402
from concourse import mybir  # noqa: E402
from concourse.bass_utils import run_bass_kernel_spmd  # noqa: E402

F32 = mybir.dt.float32
BF16 = mybir.dt.bfloat16
FP8 = mybir.dt.float8e4
AF = mybir.ActivationFunctionType
ALU = mybir.AluOpType
DR = mybir.MatmulPerfMode.DoubleRow

N_CORES = 8
MARGIN = 0.5
EPS = 1e-8
CH = 384          # gram tile free dim (half a 768 slab)
RT = 6            # 128-row tiles per slab
KT = 8            # contraction k-tiles (D/128)


def _chunk_list():
    """Per-core chunk/tile schedule (identical on every core).

    Returns list of (slot, ch, [(rt, mask_m_or_None), ...]).
    slot 0 = own slab; slot s>0 = slab of core (j+s) mod 8.
    """
    tri_ch0 = [(0, 0), (1, 1), (2, 2)]
    tri_ch1 = [(0, None), (1, None), (2, None), (3, 0), (4, 1), (5, 2)]
    full = [(rt, None) for rt in range(RT)]
    chunks = []
    chunks.append((0, 0, tri_ch0))
    chunks.append((0, 1, tri_ch1))
    for s in (1, 2, 3):
        chunks.append((s, 0, full))
    chunks.append((4, 0, tri_ch0))
    for s in (1, 2, 3):
        chunks.append((s, 1, full))
    chunks.append((4, 1, tri_ch1))
    return chunks


def build_program(B, D):
    N = 3 * B
    LOCC = N // N_CORES
    assert D == KT * 128 and LOCC == 2 * CH and LOCC == RT * 128

    nc = bacc.Bacc(
        "TRN2",
        target_bir_lowering=False,
        debug=False,
        num_devices=N_CORES,
    )

    xt_in = nc.dram_tensor("xt_in", [D, LOCC], BF16, kind="ExternalInput")
    u_in = nc.dram_tensor("u_in", [RT, 128, 4], BF16, kind="ExternalInput")
    chunks = _chunk_list()
    NCHUNK = len(chunks)
    NTILE = sum(len(c[2]) for c in chunks)
    accm_out = nc.dram_tensor("accm_out", [4, NCHUNK * CH], F32,
                              kind="ExternalOutput")
    asums_out = nc.dram_tensor("asums_out", [128, NTILE], F32,
                               kind="ExternalOutput")

    with tile.TileContext(nc) as tc:
        with (
            tc.tile_pool(name="persist", bufs=1) as persist,
            tc.tile_pool(name="work", bufs=3) as work,
            tc.tile_pool(name="dram", bufs=1, space="DRAM") as dram,
            tc.tile_pool(name="psum_g", bufs=3, space="PSUM") as psum_g,
            tc.tile_pool(name="psum_a", bufs=2, space="PSUM") as psum_a,
        ):
            pid = nc.sync.partition_id()

            # warm-up collective: pays the one-time mesh-setup latency
            # concurrently with the norm phase, so the real all-gathers
            # below start data-bound
            warm_s = persist.tile([1, 4], F32, tag="warm")
            nc.gpsimd.memset(warm_s[:], 0.0)
            warm_in = dram.tile([1, 4], F32, tag="warm_in")
            warm_out = dram.tile([N_CORES, 4], F32, tag="warm_out",
                                 addr_space="Shared")
            nc.sync.dma_start(warm_in[:], warm_s[:])
            nc.gpsimd.collective_compute(
                "AllGather",
                ALU.bypass,
                replica_groups=[list(range(N_CORES))],
                ins=[warm_in[:].opt()],
                outs=[warm_out[:].opt()],
            )

            # ---- constants ----
            ones_col = persist.tile([128, 1], F32, tag="ones_col")
            nc.gpsimd.memset(ones_col[:], 1.0)
            ones_row = persist.tile([1, 128], F32, tag="ones_row")
            nc.gpsimd.memset(ones_row[:], 1.0)
            iota_i = persist.tile([128, CH], mybir.dt.int32, tag="iota_i")
            nc.gpsimd.iota(iota_i[:], [[1, CH]], base=0, channel_multiplier=-1)
            iota_t = persist.tile([128, CH], F32, tag="iota")
            nc.scalar.copy(iota_t[:], iota_i[:])
            wmask = []
            for m in range(3):
                w_m = persist.tile([128, CH], BF16, tag=f"w{m}", name=f"w{m}")
                nc.vector.tensor_scalar(w_m[:], iota_t[:], float(1 - 128 * m),
                                        0.0, ALU.add, ALU.max)
                nc.vector.tensor_scalar_min(w_m[:], w_m[:], 2.0)
                wmask.append(w_m)
            u1_s = persist.tile([128, RT, 4], BF16, tag="u1")
            nc.sync.dma_start(u1_s[:], u_in[:].rearrange("r p c -> p r c"))
            u2_s = persist.tile([128, RT, 4], BF16, tag="u2")
            nc.vector.tensor_scalar_mul(u2_s[:], u1_s[:], 2.0)

            # ---- phase 1: load slab, compute norms, scale, cast fp8 ----
            xb = [persist.tile([128, LOCC], BF16, tag=f"xb{t}", name=f"xb{t}")
                  for t in range(KT)]
            for t in range(KT):
                nc.sync.dma_start(xb[t][:], xt_in[t * 128:(t + 1) * 128, :])
            with (
                tc.tile_pool(name="sq", bufs=3) as sq_pool,
                tc.tile_pool(name="psum_n", bufs=1, space="PSUM") as psum_n,
                tc.tile_pool(name="psum_b", bufs=1, space="PSUM") as psum_b,
            ):
                ss_ps = [psum_n.tile([1, CH], F32, tag=f"ss{h}", name=f"ss{h}")
                         for h in range(2)]
                for t in range(KT):
                    sq = sq_pool.tile([128, LOCC], F32, tag="sq")
                    eng = (nc.scalar, nc.vector, nc.gpsimd)[t % 3]
                    if eng is nc.scalar:
                        nc.scalar.activation(sq[:], xb[t][:], AF.Square)
                    else:
                        eng.tensor_tensor(sq[:], xb[t][:], xb[t][:], ALU.mult)
                    for h in range(2):
                        nc.tensor.matmul(
                            ss_ps[h][:], ones_col[:],
                            sq[:, h * CH:(h + 1) * CH],
                            start=(t == 0), stop=(t == KT - 1),
                        )
                ss_s = persist.tile([1, LOCC], F32, tag="ss_s")
                for h in range(2):
                    nc.scalar.copy(ss_s[:, h * CH:(h + 1) * CH], ss_ps[h][:])
                # broadcast to 128 partitions, then rsqrt at full width
                ssb = persist.tile([128, LOCC], F32, tag="ssb")
                for h in range(2):
                    bc = psum_b.tile([128, CH], F32, tag="bc")
                    nc.tensor.matmul(bc[:], ones_row[:],
                                     ss_s[:, h * CH:(h + 1) * CH],
                                     start=True, stop=True)
                    nc.scalar.copy(ssb[:, h * CH:(h + 1) * CH], bc[:])
                nc.vector.tensor_scalar_max(ssb[:], ssb[:], float(EPS * EPS))
                lnb = persist.tile([128, LOCC], F32, tag="lnb")
                nc.scalar.activation(lnb[:], ssb[:], AF.Ln)
                inv_b = persist.tile([128, LOCC], F32, tag="inv_b")
                nc.scalar.activation(inv_b[:], lnb[:], AF.Exp, scale=-0.5)

            # normalized fp8 slab, split into the two 384-col halves
            xn8 = [persist.tile([128, KT, CH], FP8, tag=f"xn8{c}",
                                name=f"xn8{c}") for c in range(2)]
            for c in range(2):
                for t in range(KT):
                    eng = (nc.vector, nc.gpsimd)[t % 2]
                    eng.tensor_tensor(
                        xn8[c][:, t, :],
                        xb[t][:, c * CH:(c + 1) * CH],
                        inv_b[:, c * CH:(c + 1) * CH],
                        ALU.mult,
                    )

            # ---- phase 2: all-gather fp8 slabs (two column-half chunks);
            # rotated slot placement via per-core branched DMA-in, emitted
            # lazily per (slot,ch) group inside the gram loop ----
            xtf8 = [[persist.tile([128, KT, CH], FP8, tag=f"xtf8_{s}_{c}",
                                  name=f"xtf8_{s}_{c}") for c in range(2)]
                    for s in range(4)]
            ag_in = [dram.tile([D, CH], FP8, tag=f"agi{c}", name=f"agi{c}")
                     for c in range(2)]
            ag_out = [dram.tile([N_CORES * D, CH], FP8, tag=f"ago{c}",
                                name=f"ago{c}", addr_space="Shared")
                      for c in range(2)]
            for c in range(2):
                for t in range(KT):
                    nc.sync.dma_start(ag_in[c][t * 128:(t + 1) * 128, :],
                                      xn8[c][:, t, :])
            for c in range(2):
                nc.gpsimd.collective_compute(
                    "AllGather",
                    ALU.bypass,
                    replica_groups=[list(range(N_CORES))],
                    ins=[ag_in[c][:].opt()],
                    outs=[ag_out[c][:].opt()],
                )

            def fetch_half(c):
                # place slabs of cores j+1..j+4 into rotated slots (one ch)
                for j in range(N_CORES):
                    with tc.If(pid == j):
                        for s in (1, 2, 3, 4):
                            src_core = (j + s) % N_CORES
                            nc.sync.dma_start(
                                xtf8[s - 1][c][:],
                                ag_out[c][src_core * D:(src_core + 1) * D, :]
                                .rearrange("(t p) j -> p t j", p=128))

            # ---- phase 3: gram tiles + loss pieces ----
            asums = persist.tile([128, NTILE], F32, tag="asums")
            acc_sbuf = persist.tile([4, NCHUNK * CH], F32, tag="acc_sbuf")
            idx = 0
            fetched = set()
            for ci, (s, c, rts) in enumerate(chunks):
                if s > 0 and c not in fetched:
                    fetched.add(c)
                    fetch_half(c)
                acc_ps = psum_a.tile([4, CH], F32, tag="acc")
                for ti, (rt, m) in enumerate(rts):
                    g_ps = psum_g.tile([128, CH], F32, tag="g")
                    chs, u0 = rt // 3, 128 * (rt % 3)
                    for tp in range(KT // 2):
                        stat = xn8[chs][:, 2 * tp:2 * tp + 2, u0:u0 + 128]
                        if s == 0:
                            mov = xn8[c][:, 2 * tp:2 * tp + 2, :]
                        else:
                            mov = xtf8[s - 1][c][:, 2 * tp:2 * tp + 2, :]
                        nc.tensor.matmul(g_ps[:], stat, mov,
                                         start=(tp == 0), stop=(tp == 3),
                                         perf_mode=DR)
                    a_t = work.tile([128, CH], BF16, tag="A")
                    nc.scalar.activation(
                        a_t[:], g_ps[:], AF.Square, bias=1.0, scale=-1.0,
                        accum_out=(None if m is not None
                                   else asums[:, idx:idx + 1]))
                    r_t = work.tile([128, CH], BF16, tag="R")
                    nc.vector.tensor_scalar(r_t[:], g_ps[:], float(MARGIN),
                                            float(MARGIN), ALU.max,
                                            ALU.subtract)
                    r2_t = work.tile([128, CH], BF16, tag="R2")
                    nc.vector.tensor_tensor(r2_t[:], r_t[:], r_t[:], ALU.mult)
                    m_t = work.tile([128, CH], BF16, tag="M")
                    m_eng = nc.vector if idx % 5 == 4 else nc.gpsimd
                    m_eng.tensor_tensor(m_t[:], a_t[:], r2_t[:],
                                        ALU.subtract)
                    if m is not None:
                        mw_t = work.tile([128, CH], BF16, tag="MW")
                        nc.gpsimd.tensor_tensor(mw_t[:], m_t[:], wmask[m][:],
                                                ALU.mult)
                        aw_t = work.tile([128, CH], BF16, tag="AW")
                        nc.vector.tensor_tensor(aw_t[:], a_t[:], wmask[m][:],
                                                ALU.mult)
                        scr = work.tile([128, CH], BF16, tag="SCR")
                        nc.scalar.activation(scr[:], aw_t[:], AF.Copy,
                                             accum_out=asums[:, idx:idx + 1])
                        macc, u_sel = mw_t, u1_s
                    else:
                        macc, u_sel = m_t, u2_s
                    nc.tensor.matmul(acc_ps[:], u_sel[:, rt, :],
                                     macc[:], start=(ti == 0),
                                     stop=(ti == len(rts) - 1),
                                     skip_group_check=True)
                    idx += 1
                nc.vector.tensor_copy(acc_sbuf[:, ci * CH:(ci + 1) * CH],
                                      acc_ps[:])
            assert idx == NTILE
            nc.sync.dma_start(accm_out[:], acc_sbuf[:])
            nc.sync.dma_start(asums_out[:], asums[:])

    nc.compile()
    return nc


_PROGRAM_CACHE = {}


def _get_program(B, D):
    key = (B, D)
    if key not in _PROGRAM_CACHE:
        _PROGRAM_CACHE[key] = build_program(B, D)
    return _PROGRAM_CACHE[key]


def kernel(features, labels, neg_labels):
    features = np.asarray(features)
    labels = np.asarray(labels)
    neg_labels = np.asarray(neg_labels)
    B, three, D = features.shape
    assert three == 3
    N = 3 * B
    LOCC = N // N_CORES

    nc = _get_program(B, D)

    flat = features.reshape(N, D).astype(np.float32, copy=False)
    xt_full = np.ascontiguousarray(flat.T).astype(ml_dtypes.bfloat16)
    L = np.stack([labels, labels, neg_labels], axis=1).reshape(-1)

    in_maps = []
    for k in range(N_CORES):
        xt_slice = np.ascontiguousarray(xt_full[:, k * LOCC:(k + 1) * LOCC])
        lr = L[k * LOCC:(k + 1) * LOCC]
        u = (lr[:, None] == np.arange(4)[None, :]).astype(ml_dtypes.bfloat16)
        in_maps.append({
            "xt_in": xt_slice,
            "u_in": np.ascontiguousarray(u.reshape(RT, 128, 4)),
        })

    res = run_bass_kernel_spmd(nc, in_maps, list(range(N_CORES)))
    global LAST_RESULT
    LAST_RESULT = res

    chunks = _chunk_list()
    # tile weights in program order: masked tiles 1 (mask-internal), full 2
    tile_w = []
    for (_s, _c, rts) in chunks:
        for (_rt, m) in rts:
            tile_w.append(1.0 if m is not None else 2.0)
    tile_w = np.asarray(tile_w, dtype=np.float64)

    S = 0.0
    for j in range(N_CORES):
        accm = res.results[j]["accm_out"].astype(np.float64)  # [4, NCHUNK*CH]
        asums = res.results[j]["asums_out"].astype(np.float64)  # [128, NTILE]
        # class-select per global column of each chunk
        for ci, (s, c, _rts) in enumerate(chunks):
            g0 = ((j + s) % N_CORES) * LOCC + c * CH
            lcols = L[g0:g0 + CH]
            blk = accm[:, ci * CH:(ci + 1) * CH]
            S += float(blk[lcols, np.arange(CH)].sum())
        S -= float(accm.sum())
        S += float((asums.sum(axis=0) * tile_w).sum())

    P = 3 * B + 9 * B * (B - 1) // 2
    return np.float32(S / (4.0 * P))
